# revision 20
# baseline (speedup 1.0000x reference)
"""Trainium2 Bass kernel for nn_ExtractionBlock (PointNet-style extraction block).

Sharding: data-parallel over batch B=8 across 8 NeuronCores (1 point cloud/core).
Pipeline per core: FPS (sequential argmax loop) -> kNN (exact d2 + gpsimd top-k)
-> BN-folded conv/res blocks on gathered neighborhoods -> maxpool -> pos blocks.
"""
import numpy as np

import concourse.bass as bass
import concourse.bacc as bacc
import concourse.mybir as mybir
import concourse.tile as tile
from concourse import bass_utils

F32 = mybir.dt.float32
I32 = mybir.dt.int32
I16 = mybir.dt.int16
U32 = mybir.dt.uint32
ALU = mybir.AluOpType
ACTF = mybir.ActivationFunctionType

B, N, C_IN, C_OUT = 8, 8192, 64, 128
S, K = 2048, 32
NP_, NF = 128, 64  # FPS layout: point i -> partition i//64, free i%64
BIGC = float(N)    # X encoding: X = BIGC - i

NEG_BIG = -3.0e38


def _bn_fold(w, g, b, m, v):
    s = np.asarray(g, np.float64) / np.sqrt(np.asarray(v, np.float64) + 1e-5)
    s = s.astype(np.float32)
    W = (np.asarray(w, np.float32) * s[:, None]).astype(np.float32)
    t = (np.asarray(b, np.float32) - np.asarray(m, np.float32) * s).astype(np.float32)
    return W, t


def build_nc(num_steps=S, do_knn=True, do_convs=True, n_chunks=16, num_devices=8,
             use_f32r=False):
    nc = bacc.Bacc("TRN2", target_bir_lowering=False, debug=False,
                   num_devices=num_devices)
    MMDT = mybir.dt.float32r if use_f32r else F32

    # ---------------- DRAM tensors ----------------
    d_xyz = nc.dram_tensor("xyz", [N, 3], F32, kind="ExternalInput")
    d_feat = nc.dram_tensor("feat", [N, C_IN], F32, kind="ExternalInput")
    # folded weights (transposed: [c_in, c_out]) and biases
    d_wgT = nc.dram_tensor("wgT", [C_IN, C_OUT], F32, kind="ExternalInput")
    d_wcT = nc.dram_tensor("wcT", [C_IN, C_OUT], F32, kind="ExternalInput")
    d_tte = nc.dram_tensor("tte", [C_OUT, 1], F32, kind="ExternalInput")
    d_rw = {}
    for ph in ("pre", "pos"):
        for i in range(2):
            for wn in ("w1T", "w2T"):
                d_rw[f"{ph}{i}{wn}"] = nc.dram_tensor(f"{ph}{i}{wn}", [C_OUT, C_OUT], F32, kind="ExternalInput")
            for tn in ("t1", "t2"):
                d_rw[f"{ph}{i}{tn}"] = nc.dram_tensor(f"{ph}{i}{tn}", [C_OUT, 1], F32, kind="ExternalInput")
    # constants
    d_revX = nc.dram_tensor("revX", [NP_, NF], F32, kind="ExternalInput")   # 8192 - (p*64+f)
    d_ones128 = nc.dram_tensor("ones128", [NP_, NP_], F32, kind="ExternalInput")
    d_negones128 = nc.dram_tensor("negones128", [NP_, NP_], F32, kind="ExternalInput")
    d_ident = nc.dram_tensor("ident", [NP_, NP_], F32, kind="ExternalInput")
    d_negxyz0 = nc.dram_tensor("negxyz0", [NP_, 3], F32, kind="ExternalInput")  # -xyz[0] bcast
    d_xyzcols = nc.dram_tensor("xyzcols", [3, N], F32, kind="ExternalInput")

    d_newxyz = nc.dram_tensor("new_xyz", [S, 3], F32, kind="ExternalOutput")
    d_xout = nc.dram_tensor("xout", [C_OUT, S], F32, kind="ExternalOutput")
    d_fpsX = nc.dram_tensor("fpsX", [1, S], F32, kind="ExternalOutput")  # debug: X=8192-idx
    d_knn = nc.dram_tensor("knn", [S, K], U32, kind="ExternalOutput")    # debug idxs
    d_kni16 = nc.dram_tensor("kni16", [S, K], I16, kind="Internal")
    d_fpsi16 = nc.dram_tensor("fpsi16d", [1, S], I16, kind="Internal")
    d_cstn = nc.dram_tensor("cstn", [1, 3 * S], F32, kind="Internal")

    with tile.TileContext(nc) as tc:
        with (
            tc.tile_pool(name="const", bufs=1) as cpool,
            tc.tile_pool(name="big", bufs=1) as bpool,
            tc.tile_pool(name="fps", bufs=1) as fpool,
            tc.tile_pool(name="work", bufs=2) as wpool,
            tc.tile_pool(name="ps", bufs=2, space="PSUM") as pspool,
            tc.tile_pool(name="psbig", bufs=2, space="PSUM") as psbig,
        ):
            # ---- constants to SBUF ----
            revX = cpool.tile([NP_, NF], F32, tag="revX")
            nc.sync.dma_start(revX[:, :], d_revX.ap())
            ones128 = cpool.tile([NP_, NP_], F32, tag="ones128")
            nc.sync.dma_start(ones128[:, :], d_ones128.ap())
            negones128 = cpool.tile([NP_, NP_], F32, tag="negones128")
            nc.sync.dma_start(negones128[:, :], d_negones128.ap())
            ident = cpool.tile([NP_, NP_], F32, tag="ident")
            nc.sync.dma_start(ident[:, :], d_ident.ap())

            # ---- FPS state ----
            xpl = fpool.tile([NP_, NF], F32, tag="xpl")
            ypl = fpool.tile([NP_, NF], F32, tag="ypl")
            zpl = fpool.tile([NP_, NF], F32, tag="zpl")
            xyz_r = d_xyz.ap().rearrange("(p f) c -> c p f", p=NP_)
            nc.sync.dma_start(xpl[:, :], xyz_r[0])
            nc.sync.dma_start(ypl[:, :], xyz_r[1])
            nc.sync.dma_start(zpl[:, :], xyz_r[2])
            dists = fpool.tile([NP_, NF], F32, tag="dists")
            nc.vector.memset(dists[:, :], 1.0e10)
            cbias = fpool.tile([NP_, 4], F32, tag="cbias")  # [-cx,-cy,-cz] cols
            nc.sync.dma_start(cbias[:, 0:3], d_negxyz0.ap())
            cplane = fpool.tile([NP_, 3], F32, tag="cplane")   # per-chunk neg centers (kNN)
            pack3 = fpool.tile([NP_, 3 * S], F32, tag="pack3")
            nc.vector.memset(pack3[:, :], 0.0)
            cstoren = pack3[0:1, :]                 # neg centers interleaved (p0: rearrange-DMA ok)
            cstore = pack3[32:33, :]                # pos centers interleaved
            Xstore = pack3[64:65, 0:S]              # X = 8192 - idx
            fpsf_v = pack3[64:65, S:2 * S]
            sqx = fpool.tile([NP_, NF], F32, tag="sqx")
            sqy = fpool.tile([NP_, NF], F32, tag="sqy")
            sqz = fpool.tile([NP_, NF], F32, tag="sqz")
            t12 = fpool.tile([NP_, NF], F32, tag="t12")
            dcur = fpool.tile([NP_, NF], F32, tag="dcur")
            eqm = fpool.tile([NP_, NF], F32, tag="eqm")
            junk = fpool.tile([NP_, NF], F32, tag="junk")
            trip = fpool.tile([NP_, 2], F32, tag="trip")
            gm = fpool.tile([1, 1], F32, tag="gm")
            candX = fpool.tile([1, NP_], F32, tag="candX")
            gX = fpool.tile([1, 1], F32, tag="gX")
            onehot = fpool.tile([NP_, NF], F32, tag="onehot")
            sxyz = fpool.tile([NP_, 3], F32, tag="sxyz")

            # store first center (index 0) info
            # cstore[0,0:3] = xyz[0] = -negxyz0 row0 ; Xstore[0,0] = 8192 - 0
            nc.scalar.activation(cstore[0:1, 0:3], cbias[0:1, 0:3], ACTF.Copy, scale=-1.0)
            nc.vector.memset(Xstore[0:1, 0:1], BIGC)
            nc.scalar.activation(cstoren[0:1, 0:3], cbias[0:1, 0:3], ACTF.Copy)

            for t in range(1, num_steps):
                # distance update for center selected at step t-1 (bias in cbias)
                nc.scalar.activation(sqx[:, :], xpl[:, :], ACTF.Square, bias=cbias[:, 0:1])
                nc.scalar.activation(sqy[:, :], ypl[:, :], ACTF.Square, bias=cbias[:, 1:2])
                nc.scalar.activation(sqz[:, :], zpl[:, :], ACTF.Square, bias=cbias[:, 2:3])
                nc.vector.tensor_tensor(t12[:, :], sqx[:, :], sqy[:, :], ALU.add)
                nc.vector.tensor_tensor(dcur[:, :], t12[:, :], sqz[:, :], ALU.add)
                # dists = min(dists, d); rmax per row
                nc.vector.tensor_tensor(dists[:, :], dists[:, :], dcur[:, :], ALU.min)
                nc.vector.tensor_reduce(trip[:, 0:1], dists[:, :], mybir.AxisListType.X, ALU.max)
                # row argmax -> X of first row max
                nc.vector.tensor_scalar(eqm[:, :], dists[:, :], trip[:, 0:1], None, ALU.is_ge)
                nc.vector.tensor_tensor(junk[:, :], eqm[:, :], revX[:, :], ALU.mult)
                nc.vector.tensor_reduce(trip[:, 1:2], junk[:, :], mybir.AxisListType.X, ALU.max)
                # cross-partition: transpose cols separately (partition-0 access only)
                trM = pspool.tile([1, NP_], F32, tag="fps_ps", name="trM")
                nc.tensor.transpose(trM[:, :], trip[:, 0:1], ident[:, :])
                trX = pspool.tile([1, NP_], F32, tag="fps_psb", name="trX")
                nc.tensor.transpose(trX[:, :], trip[:, 1:2], ident[:, :])
                trXs = fpool.tile([1, NP_], F32, tag="trXs", name="trXs")
                nc.scalar.copy(trXs[0:1, :], trX[0:1, :])
                nc.vector.tensor_reduce(gm[0:1, 0:1], trM[0:1, :], mybir.AxisListType.X, ALU.max)
                nc.vector.scalar_tensor_tensor(
                    candX[0:1, :], trM[0:1, :], gm[0:1, 0:1], trXs[0:1, :],
                    ALU.is_ge, ALU.mult)
                nc.vector.tensor_reduce(gX[0:1, 0:1], candX[0:1, :], mybir.AxisListType.X, ALU.max)
                # record X (debug + fps idx recovery)
                nc.scalar.copy(Xstore[0:1, t:t + 1], gX[0:1, 0:1])
                # broadcast gX to [128,1] then onehot over revX
                gXb = pspool.tile([NP_, 1], F32, tag="fps_ps", name="gXb")
                nc.tensor.matmul(gXb[:, :], ones128[0:1, :], gX[0:1, 0:1])
                nc.vector.tensor_scalar(onehot[:, :], revX[:, :], gXb[:, 0:1], None, ALU.is_equal)
                # coords of selected point: sxyz[p, c] = sum_f onehot*plane
                nc.vector.tensor_tensor(junk[:, :], onehot[:, :], xpl[:, :], ALU.mult)
                nc.vector.tensor_reduce(sxyz[:, 0:1], junk[:, :], mybir.AxisListType.X, ALU.add)
                nc.vector.tensor_tensor(junk[:, :], onehot[:, :], ypl[:, :], ALU.mult)
                nc.vector.tensor_reduce(sxyz[:, 1:2], junk[:, :], mybir.AxisListType.X, ALU.add)
                nc.vector.tensor_tensor(junk[:, :], onehot[:, :], zpl[:, :], ALU.mult)
                nc.vector.tensor_reduce(sxyz[:, 2:3], junk[:, :], mybir.AxisListType.X, ALU.add)
                # -coords broadcast to all partitions: cb = negones.T @ sxyz
                cb = pspool.tile([NP_, 3], F32, tag="fps_ps", name="cb")
                nc.tensor.matmul(cb[:, :], negones128[:, :], sxyz[:, :])
                nc.vector.tensor_copy(cbias[:, 0:3], cb[:, :])
                # stores (off critical path)
                nc.scalar.activation(cstore[0:1, 3 * t:3 * t + 3], cb[0:1, :], ACTF.Copy, scale=-1.0)
                nc.scalar.activation(cstoren[0:1, 3 * t:3 * t + 3], cb[0:1, :], ACTF.Copy)

            # write outputs of FPS
            nc.sync.dma_start(d_newxyz.ap().rearrange("s c -> () (s c)")[0:1, 0:3 * num_steps], cstore[0:1, 0:3 * num_steps])
            nc.sync.dma_start(d_cstn.ap(), cstoren[0:1, :])
            nc.sync.dma_start(d_fpsX.ap()[0:1, 0:num_steps], Xstore[0:1, 0:num_steps])

            if not do_knn:
                # dummy-touch remaining outputs so they exist
                nc.sync.dma_start(d_xout.ap()[:, 0:NF], xpl[:, :])
                nc.sync.dma_start(d_knn.ap()[0:NP_, 0:K], junk[:, 0:K].bitcast(U32))
                return nc

            # ================= kNN =================
            QN = N // 4  # quarter width
            sqq = bpool.tile([NP_, 2 * QN], F32, tag="sqq")
            vneg = bpool.tile([NP_, N], F32, tag="vneg")
            mx8 = wpool.tile([NP_, 8], F32, tag="mx8")
            ix8 = wpool.tile([NP_, 8], U32, tag="ix8")
            knnidx_u = fpool.tile([NP_, K * (S // NP_)], U32, tag="knnidx")  # per chunk cols


            for cc in range(n_chunks):
                nc.sync.dma_start(
                    cplane[:, :],
                    d_cstn.ap().rearrange("o (p c) -> o p c", c=3)[0][3 * NP_ * cc // 3:3 * NP_ * (cc + 1) // 3, :])
                ncx = cplane[:, 0:1]
                ncy = cplane[:, 1:2]
                ncz = cplane[:, 2:3]
                for h in range(4):
                    sl = slice(h * QN, (h + 1) * QN)
                    sqa = sqq[:, 0:QN]
                    sqb = sqq[:, QN:2 * QN]
                    repq = wpool.tile([NP_, QN], F32, tag="repq")
                    nc.sync.dma_start(repq[:, :], d_xyzcols.ap()[0:1, sl].broadcast_to([NP_, QN]))
                    nc.scalar.activation(sqa[:, :], repq[:, :], ACTF.Square, bias=ncx)
                    repq = wpool.tile([NP_, QN], F32, tag="repq")
                    nc.sync.dma_start(repq[:, :], d_xyzcols.ap()[1:2, sl].broadcast_to([NP_, QN]))
                    nc.scalar.activation(sqb[:, :], repq[:, :], ACTF.Square, bias=ncy)
                    nc.vector.tensor_tensor(sqa[:, :], sqa[:, :], sqb[:, :], ALU.add)
                    repq = wpool.tile([NP_, QN], F32, tag="repq")
                    nc.sync.dma_start(repq[:, :], d_xyzcols.ap()[2:3, sl].broadcast_to([NP_, QN]))
                    nc.scalar.activation(sqb[:, :], repq[:, :], ACTF.Square, bias=ncz)
                    # vneg = -(sqa + sqb) = -d2
                    nc.vector.scalar_tensor_tensor(
                        vneg[:, sl], sqa[:, :], -1.0, sqb[:, :], ALU.mult, ALU.subtract)
                # top-32 via 4 rounds of gpsimd max8
                for r in range(4):
                    nc.vector.max(mx8[:, :], vneg[:, :])
                    nc.vector.max_index(ix8[:, :], mx8[:, :], vneg[:, :])
                    nc.vector.tensor_copy(knnidx_u[:, cc * K + r * 8:cc * K + (r + 1) * 8], ix8[:, :])
                    if r < 3:
                        nc.vector.match_replace(vneg[:, :], mx8[:, :], vneg[:, :], NEG_BIG)
                nc.sync.dma_start(
                    d_knn.ap()[cc * NP_:(cc + 1) * NP_, :], knnidx_u[:, cc * K:(cc + 1) * K])

            if not do_convs:
                nc.sync.dma_start(d_xout.ap()[:, 0:NF], junk[:, :])
                return nc

            # ================= feature pipeline =================
            # featT [64, 8192] via PE transposes of feat tiles
            featT = bpool.tile([C_IN, N], F32, tag="featT")
            for j in range(N // NP_):
                ft = wpool.tile([NP_, C_IN], F32, tag="fttile")
                nc.sync.dma_start(ft[:, :], d_feat.ap()[bass.ts(j, NP_), :])
                ps = pspool.tile([C_IN, NP_], F32, tag="fps_ps", name="ftT")
                nc.tensor.transpose(ps[:, :], ft[:, :], ident[:, :])
                nc.scalar.copy(featT[:, bass.ts(j, NP_)], ps[:, :])

            # weights to SBUF
            wgT = cpool.tile([C_IN, C_OUT], F32, tag="wgT")
            nc.sync.dma_start(wgT[:, :], d_wgT.ap())
            wcT = cpool.tile([C_IN, C_OUT], F32, tag="wcT")
            nc.sync.dma_start(wcT[:, :], d_wcT.ap())
            tte = cpool.tile([C_OUT, 1], F32, tag="tte")
            nc.sync.dma_start(tte[:, :], d_tte.ap())
            rws = {}
            for key, dt_ in d_rw.items():
                shape_ = [C_OUT, C_OUT] if key.endswith("T") else [C_OUT, 1]
                rwt = cpool.tile(shape_, F32, tag=key, name=f"rw_{key}")
                rws[key] = rwt
                nc.sync.dma_start(rws[key][:, :], dt_.ap())

            # AT = (feat @ Wg).T : [128 och, 8192] = wgT.T @ featT
            AT = bpool.tile([C_OUT, N], F32, tag="vneg", name="AT")
            for j in range(N // 512):
                ps = psbig.tile([C_OUT, 512], F32, tag="cps", name="ATp")
                nc.tensor.matmul(ps[:, :], wgT[:, :].bitcast(MMDT), featT[:, bass.ts(j, 512)].bitcast(MMDT))
                if j % 2 == 0:
                    nc.vector.tensor_copy(AT[:, bass.ts(j, 512)], ps[:, :])
                else:
                    nc.scalar.copy(AT[:, bass.ts(j, 512)], ps[:, :])

            # fps idx as wrapped int16 for ap_gather: idx = 8192 - Xstore
            fpsi16 = fpool.tile([1, S], I16, tag="fpsi16")
            nc.vector.tensor_scalar(fpsf_v[0:1, :], Xstore[0:1, :], -1.0, BIGC, ALU.mult, op1=ALU.add)
            nc.vector.tensor_scalar(fpsf_v[0:1, :], fpsf_v[0:1, :], float(N - 1), None, ALU.min)
            nc.vector.tensor_copy(fpsi16[0:1, :], fpsf_v[0:1, :])
            # wrapped layout [64, S/16] replicated to 4 core-groups
            fpsw = fpool.tile([C_IN, S // 16], I16, tag="fpsw")
            nc.sync.dma_start(d_fpsi16.ap(), fpsi16[0:1, :])
            for a in range(4):
                nc.sync.dma_start(
                    fpsw[16 * a:16 * (a + 1), :],
                    d_fpsi16.ap().rearrange("o (c q) -> o q c", q=16)[0])
            # ctrfT [64, 2048] = featT[:, fps_idx]
            ctrfT = bpool.tile([C_IN, S], F32, tag="ctrfT")
            nc.gpsimd.ap_gather(ctrfT[:, :], featT[:, :], fpsw[:, :],
                                channels=C_IN, num_elems=N, d=1, num_idxs=S)
            # BcT [128, 2048] (+tte during evac)
            BcT = bpool.tile([C_OUT, S], F32, tag="BcT")
            for j in range(S // 512):
                ps = psbig.tile([C_OUT, 512], F32, tag="cps", name="BcTp")
                nc.tensor.matmul(ps[:, :], wcT[:, :].bitcast(MMDT), ctrfT[:, bass.ts(j, 512)].bitcast(MMDT))
                nc.scalar.activation(BcT[:, bass.ts(j, 512)], ps[:, :], ACTF.Identity, bias=tte[:, 0:1])

            # knn idx -> int16 wrapped per chunk [128, 256]
            kn16 = fpool.tile([NP_, K], I16, tag="kn16")
            knw = fpool.tile([NP_, NP_ * K // 16], I16, tag="knw")
            # per-chunk conv pipeline
            CH = NP_ * K  # 4096 positions per chunk
            P1 = bpool.tile([C_OUT, S], F32, tag="P1")
            if n_chunks < 16:
                nc.vector.memset(P1[:, :], 0.0)
            gA = bpool.tile([C_OUT, CH], F32, tag="gA")
            X0 = bpool.tile([C_OUT, CH], F32, tag="featT", name="X0")
            Y1 = bpool.tile([C_OUT, CH], F32, tag="sqq", name="Y1")
            for cc in range(n_chunks):
                nc.vector.tensor_copy(kn16[:, :], knnidx_u[:, cc * K:(cc + 1) * K])
                nc.sync.dma_start(d_kni16.ap()[cc * NP_:(cc + 1) * NP_, :], kn16[:, :])
                for a in range(8):
                    nc.sync.dma_start(
                        knw[16 * a:16 * (a + 1), :],
                        d_kni16.ap()[cc * NP_:(cc + 1) * NP_, :].rearrange("p k -> (p k)").rearrange("(c q) -> q c", q=16))
                nc.gpsimd.ap_gather(gA[:, :], AT[:, :].bitcast(F32), knw[:, :],
                                    channels=C_OUT, num_elems=N, d=1, num_idxs=CH)
                # X0 = relu(gA + Bc bcast)
                nc.vector.tensor_tensor(
                    X0[:, :].rearrange("c (g k) -> c g k", k=K), gA[:, :].rearrange("c (g k) -> c g k", k=K),
                    BcT[:, cc * NP_:(cc + 1) * NP_].rearrange("c g -> c g ()").broadcast_to([C_OUT, NP_, K]), ALU.add)
                nc.scalar.activation(X0[:, :], X0[:, :], ACTF.Relu)
                # two pre res blocks; rotate buffers (in, y, out)
                rot = [(X0, Y1, gA), (gA, X0, Y1)]
                for i in range(2):
                    xin, yb, xo = rot[i]
                    w1 = rws[f"pre{i}w1T"]; w2 = rws[f"pre{i}w2T"]
                    t1 = rws[f"pre{i}t1"]; t2 = rws[f"pre{i}t2"]
                    for j in range(CH // 512):
                        ps = psbig.tile([C_OUT, 512], F32, tag="cps")
                        nc.tensor.matmul(ps[:, :], w1[:, :].bitcast(MMDT), xin[:, bass.ts(j, 512)].bitcast(MMDT))
                        nc.scalar.activation(yb[:, bass.ts(j, 512)], ps[:, :], ACTF.Relu, bias=t1[:, 0:1])
                    for j in range(CH // 512):
                        ps = psbig.tile([C_OUT, 512], F32, tag="cps")
                        nc.tensor.matmul(ps[:, :], w2[:, :].bitcast(MMDT), yb[:, bass.ts(j, 512)].bitcast(MMDT), start=True, stop=False)
                        nc.tensor.matmul(ps[:, :], ident[:, :].bitcast(MMDT), xin[:, bass.ts(j, 512)].bitcast(MMDT), start=False, stop=True)
                        nc.scalar.activation(xo[:, bass.ts(j, 512)], ps[:, :], ACTF.Relu, bias=t2[:, 0:1])
                xin = Y1
                # maxpool over K -> P1 chunk
                nc.vector.tensor_reduce(
                    P1[:, cc * NP_:(cc + 1) * NP_], Y1[:, :].rearrange("c (g k) -> c g k", k=K),
                    mybir.AxisListType.X, ALU.max)

            # pos res blocks on P1 [128, 2048]
            Q1 = bpool.tile([C_OUT, S], F32, tag="ctrfT", name="Q1")
            xin = P1
            for i in range(2):
                w1 = rws[f"pos{i}w1T"]; w2 = rws[f"pos{i}w2T"]
                t1 = rws[f"pos{i}t1"]; t2 = rws[f"pos{i}t2"]
                for j in range(S // 512):
                    ps = psbig.tile([C_OUT, 512], F32, tag="cps")
                    nc.tensor.matmul(ps[:, :], w1[:, :].bitcast(MMDT), xin[:, bass.ts(j, 512)].bitcast(MMDT))
                    nc.scalar.activation(Q1[:, bass.ts(j, 512)], ps[:, :], ACTF.Relu, bias=t1[:, 0:1])
                for j in range(S // 512):
                    ps = psbig.tile([C_OUT, 512], F32, tag="cps")
                    nc.tensor.matmul(ps[:, :], w2[:, :].bitcast(MMDT), Q1[:, bass.ts(j, 512)].bitcast(MMDT), start=True, stop=False)
                    nc.tensor.matmul(ps[:, :], ident[:, :].bitcast(MMDT), xin[:, bass.ts(j, 512)].bitcast(MMDT), start=False, stop=True)
                    nc.scalar.activation(P1[:, bass.ts(j, 512)], ps[:, :], ACTF.Relu, bias=t2[:, 0:1])
                xin = P1
            nc.sync.dma_start(d_xout.ap(), P1[:, :])

    return nc


def _host_prep(inputs):
    """Fold BN into conv weights; build per-core input maps."""
    inp = {k: np.asarray(v) for k, v in inputs.items()}
    Wte, tte = _bn_fold(inp["te_w"], inp["te_g"], inp["te_b"], inp["te_m"], inp["te_v"])
    consts = {
        "wgT": np.ascontiguousarray(Wte[:, :C_IN].T),
        "wcT": np.ascontiguousarray(Wte[:, C_IN:].T),
        "tte": tte[:, None],
    }
    for ph in ("pre", "pos"):
        for i in range(2):
            W1, t1 = _bn_fold(inp[f"{ph}_w1"][i], inp[f"{ph}_g1"][i], inp[f"{ph}_b1"][i], inp[f"{ph}_m1"][i], inp[f"{ph}_v1"][i])
            W2, t2 = _bn_fold(inp[f"{ph}_w2"][i], inp[f"{ph}_g2"][i], inp[f"{ph}_b2"][i], inp[f"{ph}_m2"][i], inp[f"{ph}_v2"][i])
            consts[f"{ph}{i}w1T"] = np.ascontiguousarray(W1.T)
            consts[f"{ph}{i}w2T"] = np.ascontiguousarray(W2.T)
            consts[f"{ph}{i}t1"] = t1[:, None]
            consts[f"{ph}{i}t2"] = t2[:, None]
    p = np.arange(NP_)[:, None]
    f = np.arange(NF)[None, :]
    consts["revX"] = (BIGC - (p * NF + f)).astype(np.float32)
    consts["ones128"] = np.ones((NP_, NP_), np.float32)
    consts["negones128"] = -np.ones((NP_, NP_), np.float32)
    consts["ident"] = np.eye(NP_, dtype=np.float32)
    in_maps = []
    for b in range(B):
        m = {"xyz": np.ascontiguousarray(inp["xyz"][b]),
             "feat": np.ascontiguousarray(inp["feat"][b]),
             "negxyz0": np.broadcast_to(-inp["xyz"][b, 0], (NP_, 3)).copy(),
             "xyzcols": np.ascontiguousarray(inp["xyz"][b].T)}
        m.update({k: np.ascontiguousarray(v) for k, v in consts.items()})
        in_maps.append(m)
    return in_maps


_NC_CACHE = {}


def kernel(**inputs):
    key = "full"
    if key not in _NC_CACHE:
        ncobj = build_nc()
        ncobj.compile()
        _NC_CACHE[key] = ncobj
    ncobj = _NC_CACHE[key]
    in_maps = _host_prep(inputs)
    res = bass_utils.run_bass_kernel_spmd(ncobj, in_maps, core_ids=list(range(B)))
    new_xyz = np.stack([res.results[b]["new_xyz"] for b in range(B)])
    xout = np.stack([res.results[b]["xout"] for b in range(B)])
    return new_xyz.astype(np.float32), xout.astype(np.float32)
